# revision 5
# baseline (speedup 1.0000x reference)
"""Trainium2 Bass kernel for int8-quantized 3x3 conv with LUT-based multiply.

v2 vs the 5912ns baseline — same math (bf16 TensorEngine matmuls reproduce
the exact-product-LUT int8 conv; host does quantization, dequant scale,
bias, and the horizontal-edge corrections), restructured schedule:

 - Input: D0 (weights + x cols [0,290) = chunks 0+1's reads) is a
   HWDGE DMA issued by SP right after the entry barrier; D1 (x cols
   [290,577)) is issued by the GPSIMD/Pool engine through the SWDGE
   path, whose descriptor generation runs on the otherwise-idle Pool
   engine from ~1.05us, so D1's transfer starts right behind D0's on
   the DMA engines (~2.81us) instead of behind SP's serial HWDGE setup.
   Its completion sem lands at ~3.94us -- before the matmul stream
   reaches chunk2 -- so only D0's sem (~3.71us) gates anything.
 - 200 16-row PE warmup matmuls keep the PE exec queue backed up so
   every real matmul's SEQ issue (where the cost model samples the
   p-state ramp) lands past the 3us threshold: all 24 run at full
   clock.  The warmups drain by the time D0's sem arrives.
 - A PE drain after each chunk's closing matmul signals that chunk's
   PSUM->SBUF copy ~146ns sooner than a matmul then_inc would (drain
   sem updates skip the PE->SBUF pipeline latency).
 - Output staging is f32 (same modeled cost as bf16: the PSUM-access
   latency dominates the copies and the writeback's descriptor count is
   size-independent), so the result stays bit-exact.
   Copies alternate ACT (chunks 0,2) / DVE (chunks 1,3) so neither
   engine's queue delays the final chunk.  Engine drains after each
   copy count up one semaphore; trigger_dma at >=2 fires the pre-armed
   kv_writeback of chunks 0+1 mid-stream, and at >=4 fires the final
   chunks 2+3 writeback, whose completion notification (+900ns) ends
   the program at 5574ns (vs 5912ns baseline).
"""

import os

import numpy as np

import concourse.bass as bass
import concourse.ap as ap_mod
from concourse import mybir, library_config
from concourse.bass_utils import run_bass_kernel_spmd
from concourse.library_overlay import lower_extended_insts

N_CORES = 8
B, CIN, H, W = 4, 64, 32, 32
COUT, K = 64, 3
OH, OW = 32, 32
HS = OH // 2              # output rows per core
WCOLS = 384               # 3 pair blocks + 3 solo blocks, 64 cols each
XLEN = 577                # x elems per partition: 1 prefix pad + 18*32 flat
XBASE = WCOLS
NCOLS = WCOLS + XLEN      # 961
NCHUNK = 4
SPLITA = XBASE + 258      # D0: weights + x cols [0,258) -- chunk0 entirely,
                          # chunk1's pairs (cols <= 257); chunk1's solos
                          # (cols up to 289) wait on D1's sem instead

F32 = mybir.dt.float32
BF16 = mybir.dt.bfloat16
I32 = mybir.dt.int32

# Optional PE warmups (moving-row counts).  Empty = rely on the wait-queue
# SEQ-delay for the p-state ramp; see module docstring.
# 200 16-row warmup matmuls: enough engine-pending work that every real
# matmul's SEQ issue lands past the 3us p-state ramp (full clock), ending
# by the time the first input DMA's semaphore arrives.
WARMUPS: list[int] = [16] * 200
OSB_DT = F32

LAST_RESULTS = None  # BassKernelResults of the most recent device run


def _quantize(t):
    """Bit-exact replica of reference._quantize_int8 in numpy f32."""
    s = np.float32(np.max(np.abs(t))) / np.float32(127.0)
    q = np.clip(np.round(t / s), np.float32(-128.0), np.float32(127.0))
    return q.astype(np.float32), s


def _build_fast_program():
    """Raw-bass SPMD program (one NeuronCore's share).

    SBUF xw layout [128, 961] bf16:
      cols 0:192   pair weight blocks kw=0,1,2: hi rows = w(1,kw)^T,
                   lo rows = w(0,kw)^T  (K=128 matmuls)
      cols 192:384 solo weight blocks kw=0,1,2: partitions 0:64 =
                   w(2,kw)^T, partitions 64:128 unused (K=64 matmuls)
      cols 384:961 x data, 32-wide row-major flat (vertical pads only):
        partition 64+p ("lo"): x col c = [0, Pflat][c]
        partition p    ("hi"): x col c = Pflat[c+31]  (shifted 32)

    Chunk q (flat positions 128q..128q+127): stationary view offset
    (from XBASE+128q) is kw for pairs (K=128), 32+kw for solos (K=64,
    partitions 0:64).  Horizontal-edge taps wrap into adjacent rows; the
    host subtracts those terms exactly.
    """
    nc = bass.Bass()
    xw_d = nc.dram_tensor("xw", [128, NCOLS], BF16, kind="ExternalInput")
    out_d = nc.dram_tensor(
        "out", [1, 128, 1, NCHUNK * COUT], OSB_DT, kind="ExternalOutput"
    )

    with (
        nc.sbuf_tensor([128, NCOLS], BF16) as xw,
        nc.sbuf_tensor([128, NCHUNK * COUT], OSB_DT) as osb,
        nc.sbuf_tensor([128, 1], I32) as ctx0,
        nc.sbuf_tensor([128, 1], I32) as ctx1,
        nc.psum_tensor([128, COUT], F32) as acc0,
        nc.psum_tensor([128, COUT], F32) as acc1,
        nc.psum_tensor([128, COUT], F32) as acc2,
        nc.psum_tensor([128, COUT], F32) as acc3,
        nc.psum_tensor([1, 512], F32) as warm,
        nc.semaphore() as sem_a,
        nc.semaphore() as sem_b,
        nc.semaphore() as prep_done,
        nc.semaphore() as c0s,
        nc.semaphore() as c1s,
        nc.semaphore() as c2s,
        nc.semaphore() as c3s,
        nc.semaphore() as copy_all,
        nc.semaphore() as dma_out,
        nc.Block(no_gpsimd_drain=True) as block,
    ):
        def xv(off):
            # [128, 128] single-free-dim stationary view of the x region
            return ap_mod.AP(xw, XBASE + off, [[NCOLS, 128], [1, 128]])

        @block.sync
        def _(sync):
            sync.dma_start(xw[:, 0:SPLITA], xw_d[:, 0:SPLITA]).then_inc(sem_a, 16)

        @block.tensor
        def _(tensor):
            if WARMUPS:
                ones = nc.const_aps.tensor(1.0, (128, 1), BF16)
                for n in WARMUPS:
                    nc.tensor.matmul(
                        warm[0:1, 0:n], ones, ones.to_broadcast((128, n)),
                        start=True, stop=True
                    )
            accs = [acc0, acc1, acc2, acc3]
            csems = [c0s, c1s, c2s, c3s]
            for q in range(NCHUNK):
                o = accs[q][:]
                base = 128 * q
                mm = nc.tensor.matmul(o, xv(base + 0), xw[:, 0:64],
                                      start=True, stop=False)
                if q == 0:
                    mm._wait_ge(sem_a, 16)
                elif q == 2:
                    mm._wait_ge(sem_b, 16)
                nc.tensor.matmul(o, xv(base + 1), xw[:, 64:128],
                                 start=False, stop=False)
                nc.tensor.matmul(o, xv(base + 2), xw[:, 128:192],
                                 start=False, stop=False)
                mm = nc.tensor.matmul(o, xv(base + 32), xw[:, 192:256],
                                      start=False, stop=False)
                if q == 1:
                    # chunk1 solos read x cols [160,290); cols >= 258 are D1's
                    mm._wait_ge(sem_b, 16)
                nc.tensor.matmul(o, xv(base + 33), xw[:, 256:320],
                                 start=False, stop=False)
                nc.tensor.matmul(o, xv(base + 34), xw[:, 320:384],
                                 start=False, stop=True)
                # drain-gate: the copy sem fires without the ~146ns
                # PE->SBUF pipeline latency a matmul then_inc would pay
                tensor.drain().then_inc(csems[q], 1)

        @block.vector
        def _(vector):
            vector.tensor_copy(osb[:, 64:128], acc1[:])._wait_ge(c1s, 1)
            vector.drain().then_inc(copy_all, 1)
            vector.tensor_copy(osb[:, 192:256], acc3[:])._wait_ge(c3s, 1)
            vector.drain().then_inc(copy_all, 1)

        @block.scalar
        def _(scalar):
            scalar.copy(osb[:, 0:64], acc0[:])._wait_ge(c0s, 1)
            scalar.drain().then_inc(copy_all, 1)
            scalar.copy(osb[:, 128:192], acc2[:])._wait_ge(c2s, 1)
            scalar.drain().then_inc(copy_all, 1)

        @block.gpsimd
        def _(gpsimd):
            # D1 via the Pool SWDGE path: desc-gen runs on the otherwise-idle
            # Pool engine starting ~1.05us, so the transfer starts right after
            # D0's (~2.81us) instead of behind SP's serial HWDGE setup
            # (~2.98us) -- sem_b lands ~140ns earlier.
            gpsimd.dma_start(xw[:, SPLITA:], xw_d[:, SPLITA:]).then_inc(sem_b, 16)
            gpsimd.load_library(library_config.attn)
            gpsimd.memset(ctx0[:], 0)
            gpsimd.memset(ctx1[:], 128)
            in0 = ap_mod.AP(osb, 0, [[256, 128], [128, 1], [128, 1], [1, 128]])
            in1 = ap_mod.AP(osb, 128, [[256, 128], [128, 1], [128, 1], [1, 128]])
            gpsimd.kv_writeback(
                out_d[:], in0, ctx0[:], prepare_only=True, sem=dma_out
            ).then_inc(prep_done, 1)
            gpsimd.kv_writeback(
                out_d[:], in1, ctx1[:], prepare_only=True, sem=dma_out
            ).then_inc(prep_done, 1)
            gpsimd.wait_ge(prep_done, 2)
            gpsimd.trigger_dma(count=1)._wait_ge(copy_all, 2)  # wb0: chunks 0+1
            gpsimd.trigger_dma(count=1)._wait_ge(copy_all, 4)  # wb1: chunks 2+3

    lower_extended_insts(nc)
    return nc


def _host_inputs(xq, wq):
    """Build the per-core input maps (row-shifted x copies + packed weights)."""
    bf = mybir.dt.np(BF16)
    xpad = np.zeros((B, CIN, H + 2, W), dtype=np.float32)
    xpad[:, :, 1 : H + 1, :] = xq  # vertical pads only; 32 cols

    def wT(kh, kw):
        return wq[:, :, kh, kw].T  # [CIN, COUT]

    wcat = np.zeros((128, WCOLS), dtype=np.float32)
    for kw in range(3):
        wcat[0:CIN, 64 * kw : 64 * (kw + 1)] = wT(1, kw)
        wcat[CIN:, 64 * kw : 64 * (kw + 1)] = wT(0, kw)
        wcat[0:CIN, 192 + 64 * kw : 192 + 64 * (kw + 1)] = wT(2, kw)
    wcat_bf = wcat.astype(bf)

    in_maps = []
    for c in range(N_CORES):
        b, hh = divmod(c, 2)
        sl = xpad[b, :, hh * HS : hh * HS + HS + 2, :]  # [CIN, 18, 32]
        pflat = sl.reshape(CIN, 18 * 32)                # 576 flat elems
        xw = np.zeros((128, NCOLS), dtype=bf)
        xw[:, 0:WCOLS] = wcat_bf
        # lo copy: [0, Pflat[0:576]]
        xw[CIN:, XBASE + 1 : XBASE + 577] = pflat.astype(bf)
        # hi copy: lo shifted by 32 -> Pflat[31:576], zero-padded
        xw[0:CIN, XBASE : XBASE + 545] = pflat[:, 31:576].astype(bf)
        in_maps.append({"xw": xw})
    return in_maps


def _run_fast(xq, sx, wq, sw, bias):
    in_maps = _host_inputs(xq, wq)
    nc = _build_fast_program()
    global LAST_RESULTS
    res = run_bass_kernel_spmd(
        nc,
        in_maps,
        list(range(N_CORES)),
        trace=bool(int(os.environ.get("KERNEL_TRACE", "0"))),
    )
    LAST_RESULTS = res

    s = np.float32(sx) * np.float32(sw)
    xpad = np.zeros((B, CIN, H + 2, W), dtype=np.float64)
    xpad[:, :, 1 : H + 1, :] = xq
    kh = np.arange(K)
    r = np.arange(HS)
    w0 = wq[:, :, :, 0].astype(np.float64)  # [o, ch, kh]
    w2 = wq[:, :, :, 2].astype(np.float64)
    out = np.empty((B, COUT, OH, OW), dtype=np.float32)
    for c in range(N_CORES):
        b, hh = divmod(c, 2)
        sl = xpad[b, :, hh * HS : hh * HS + HS + 2, :]  # [CIN, 18, 32]
        dev = (
            res.results[c]["out"].astype(np.float32).reshape(128, NCHUNK, COUT)
        )
        raw = (
            dev.transpose(1, 0, 2).reshape(NCHUNK * 128, COUT)
            .reshape(HS, 32, COUT).astype(np.float64)
        )  # [r, c, o]
        # Edge corrections: taps that wrapped into neighboring rows.
        ER = np.zeros((CIN, 19))          # ER[ch, j+1] = Pflat[ch, j, 31]
        ER[:, 1:] = sl[:, :, 31]
        EL = np.zeros((CIN, 19))          # EL[ch, j] = Pflat[ch, j, 0]
        EL[:, :18] = sl[:, :, 0]
        cr = np.einsum("ock,crk->ro", w0, ER[:, r[:, None] + kh[None, :]])
        cl = np.einsum("ock,crk->ro", w2, EL[:, r[:, None] + kh[None, :] + 1])
        raw[:, 0, :] -= cr
        raw[:, 31, :] -= cl
        out[b, :, hh * HS : (hh + 1) * HS, :] = (
            raw.astype(np.float32).transpose(2, 0, 1) * s
            + bias[:, None, None].astype(np.float32)
        )
    return out


def _run_generic(xq, sx, wq, sw, lut, bias):
    """Arbitrary-LUT path: faithful gather-accumulate (host-side)."""
    ixpad = np.full((B, CIN, H + 2, W + 2), 128, dtype=np.int64)
    ixpad[:, :, 1 : H + 1, 1 : W + 1] = xq.astype(np.int64) + 128
    iw = wq.reshape(COUT, CIN, K * K).astype(np.int64) + 128  # [o, ci, pos]

    acc = np.zeros((B, COUT, OH, OW), dtype=np.float32)
    for ci in range(CIN):
        for p in range(K * K):
            kh, kw = divmod(p, K)
            ixs = ixpad[:, ci, kh : kh + OH, kw : kw + OW]      # [B, OH, OW]
            rows = lut[ixs]                                      # [B, OH, OW, 256]
            contrib = rows[..., iw[:, ci, p]]                    # [B, OH, OW, COUT]
            acc += contrib.transpose(0, 3, 1, 2)
    out = acc * (np.float32(sx) * np.float32(sw))
    return out + bias.reshape(1, COUT, 1, 1)


def kernel(x, weight, lut=None, gradient_lut=None, bias=None):
    x = np.asarray(x, dtype=np.float32)
    weight = np.asarray(weight, dtype=np.float32)
    lut = np.asarray(lut, dtype=np.float32)
    bias = np.asarray(bias, dtype=np.float32)

    xq, sx = _quantize(x)
    wq, sw = _quantize(weight)

    q = np.arange(-128, 128, dtype=np.float32)
    if np.array_equal(lut, np.outer(q, q)):
        return _run_fast(xq, sx, wq, sw, bias)
    return _run_generic(xq, sx, wq, sw, lut, bias)
